# revision 8
# baseline (speedup 1.0000x reference)
"""DeepSeekV3 token-choice top-k MoE router on 8 Trainium2 NeuronCores.

kernel(x, gate, bias) -> (scores_per_expert [131072] f32,
                          token_idxs_experts_sorted [131072] i32,
                          num_tokens_per_expert [256] i32)

Sharding: x [16384, 7168] split into 8 token shards of 2048; gate/bias
replicated. Each core computes, for its tokens, the dense routing map
rn [2048, 256] f32 (2.5 * normalized unbiased score at the 8 selected
experts, exactly 0 elsewhere). Host gathers shards and assembles the
expert-sorted outputs (stable order == expert-major, token-ascending,
which np.nonzero on the transposed mask yields directly).

Per-core kernel:
  - logitsT[e, t] = gate @ x_shard.T accumulated in PSUM via bf16x3
    (x = xh + xl, gate = gh + gl in bf16; terms xh@gh + xh@gl + xl@gh;
    products are exact in fp32 PSUM, dropped xl@gl ~2^-18 — verified to
    reproduce fp32 top-k selections exactly on this input).
  - x transposed on-chip: 32x32-block-strided DMA (128B runs) + DVE
    32x32 block transpose; bf16 split on GPSIMD (hi) + DVE (lo).
  - Gate is the PE stationary operand: one ldweights per (chunk, half,
    term) reused across 4 moving tiles of 512 tokens. PSUM = 8 banks.
  - sigma = 1/(1 + exp(-l)): ACT Exp (<=2 ulp) + DVE reciprocal.
  - Group-limited top-8 via DVE max8: top-2-per-group sums -> group
    scores; top-4 group threshold -> -1e30 mask; top-8 threshold (8th
    max) -> selection mask; normalize, scale by 2.5.
"""
import numpy as np

N_TOKENS, DIM, NUM_EXPERTS, N_CORES = 16384, 7168, 256, 8
T = N_TOKENS // N_CORES

_NC_CACHE = {}


def _build():
    from contextlib import ExitStack
    import concourse.bacc as bacc
    import concourse.tile as tile
    from concourse import mybir

    F32 = mybir.dt.float32
    BF16 = mybir.dt.bfloat16
    I32 = mybir.dt.int32
    AF = mybir.ActivationFunctionType
    OP = mybir.AluOpType
    NEG_BIG = -1.0e30

    D, E = DIM, NUM_EXPERTS
    KC = D // 128
    NT4 = T // 512
    NT1 = T // 128
    EH = E // 128
    G = 8
    EPG = E // G

    nc = bacc.Bacc("TRN2", target_bir_lowering=False, debug=False,
                   enable_asserts=True)
    x_d = nc.dram_tensor("x", [T, D], F32, kind="ExternalInput")
    g_d = nc.dram_tensor("gate", [E, D], F32, kind="ExternalInput")
    b_d = nc.dram_tensor("bias", [E], F32, kind="ExternalInput")
    rn_d = nc.dram_tensor("rn", [T, E], F32, kind="ExternalOutput")

    with tile.TileContext(nc) as tc, ExitStack() as ctx:
        const = ctx.enter_context(tc.tile_pool(name="const", bufs=1))
        gpool = ctx.enter_context(tc.tile_pool(name="gpool", bufs=1))
        xpool = ctx.enter_context(tc.tile_pool(name="xpool", bufs=4))
        upool = ctx.enter_context(tc.tile_pool(name="upool", bufs=1))
        spool = ctx.enter_context(tc.tile_pool(name="spool", bufs=2))

        bias_bc = const.tile([128, E], F32, name="bias_bc")
        nc.sync.dma_start(bias_bc[0:1, :], b_d[:].unsqueeze(0))
        nc.gpsimd.partition_broadcast(bias_bc[:], bias_bc[0:1, :])

        iot = const.tile([128, 128], I32, name="iot")
        nc.gpsimd.iota(iot[:], pattern=[[-1, 128]], base=0, channel_multiplier=1)
        ident = const.tile([128, 128], F32, name="ident")
        nc.vector.tensor_single_scalar(ident[:], iot[:], 0, OP.is_equal)

        # gate: block-strided load + 32x32 transpose -> gT [dim, e]; bf16 split
        ghs, gls = [], []
        for k in range(KC):
            ga = gpool.tile([128, E], F32, name=f"ga{k}", tag="ga")
            for a in range(4):
                asrc = g_d[:, k * 128 + a * 32:k * 128 + (a + 1) * 32].rearrange(
                    "(b i) j -> i b j", i=32)
                eng_g = nc.scalar if a >= 2 else nc.sync
                eng_g.dma_start(
                    ga[a * 32:(a + 1) * 32, :].rearrange("p (b j) -> p b j", j=32),
                    asrc)
            gt = gpool.tile([128, E], F32, name=f"gt{k}", tag="gt")
            nc.vector.transpose(gt[:], ga[:])
            gh = gpool.tile([128, E], BF16, name=f"gh{k}", tag=f"gh{k}")
            nc.vector.tensor_copy(gh[:], gt[:])
            gl = gpool.tile([128, E], BF16, name=f"gl{k}", tag=f"gl{k}")
            nc.vector.tensor_sub(gl[:], gt[:], gh[:])
            ghs.append(gh); gls.append(gl)

        ps_ctx = tc.tile_pool(name="ps", bufs=1, space="PSUM")
        ps = ps_ctx.__enter__()
        psums = [ps.tile([128, 512], F32, name=f"psum{h}_{t}", tag=f"psum{h}_{t}")
                 for h in range(EH) for t in range(NT4)]

        for k in range(KC):
            xa = xpool.tile([128, T], F32, name="xa", tag="xa")
            for a in range(4):
                asrc = x_d[:, k * 128 + a * 32:k * 128 + (a + 1) * 32].rearrange(
                    "(b i) j -> i b j", i=32)
                eng = (nc.sync, nc.sync, nc.scalar, nc.gpsimd)[a]
                eng.dma_start(
                    xa[a * 32:(a + 1) * 32, :].rearrange("p (b j) -> p b j", j=32),
                    asrc)
            xt = xpool.tile([128, T], F32, name="xt", tag="xt")
            nc.vector.transpose(xt[:], xa[:])
            xh = xpool.tile([128, T], BF16, name="xh", tag="xh")
            nc.scalar.copy(xh[:], xt[:])
            xl = xpool.tile([128, T], BF16, name="xl", tag="xl")
            nc.vector.tensor_sub(xl[:], xt[:], xh[:])

            for h in range(EH):
                wh = ghs[k][:, h * 128:(h + 1) * 128]
                nc.tensor.ldweights(wh)
                for xw, first in ((xh, True), (xl, False)):
                    for t in range(NT4):
                        nc.tensor.matmul(
                            psums[h * NT4 + t][:], wh, xw[:, t * 512:(t + 1) * 512],
                            start=(k == 0 and first),
                            stop=False)
                wl = gls[k][:, h * 128:(h + 1) * 128]
                nc.tensor.ldweights(wl)
                for t in range(NT4):
                    nc.tensor.matmul(
                        psums[h * NT4 + t][:], wl, xh[:, t * 512:(t + 1) * 512],
                        start=False,
                        stop=(k == KC - 1))

        uTs = []
        for h in range(EH):
            uT = upool.tile([128, T], F32, name=f"uT{h}", tag=f"uT{h}")
            for t in range(NT4):
                nc.scalar.activation(uT[:, t * 512:(t + 1) * 512],
                                     psums[h * NT4 + t][:], AF.Exp, scale=-1.0)
            uTs.append(uT)
        ps_ctx.__exit__(None, None, None)
        pst_ctx = tc.tile_pool(name="pst", bufs=2, space="PSUM")
        pst = pst_ctx.__enter__()

        for tt in range(NT1):
            u = spool.tile([128, E], F32, name="u", tag="u")
            for h in range(EH):
                ptr = pst.tile([128, 128], F32, name="ptr", tag="ptr")
                nc.tensor.transpose(ptr[:], uTs[h][:, tt * 128:(tt + 1) * 128],
                                    ident[:])
                nc.vector.tensor_copy(u[:, h * 128:(h + 1) * 128], ptr[:])

            sig = spool.tile([128, E], F32, name="sig", tag="sig")
            nc.vector.tensor_scalar_add(sig[:], u[:], 1.0)
            nc.vector.reciprocal(sig[:], sig[:])
            sfc = spool.tile([128, E], F32, name="sfc", tag="sfc")
            nc.vector.tensor_add(sfc[:], sig[:], bias_bc[:])

            m8 = spool.tile([128, G, 8], F32, name="m8", tag="m8")
            for g in range(G):
                nc.vector.max(m8[:, g, :], sfc[:, g * EPG:(g + 1) * EPG])
            gs = spool.tile([128, G], F32, name="gs", tag="gs")
            nc.vector.tensor_add(gs[:], m8[:, :, 0], m8[:, :, 1])
            gs8 = spool.tile([128, 8], F32, name="gs8", tag="gs8")
            nc.vector.max(gs8[:], gs[:])
            gmneg = spool.tile([128, G], F32, name="gmneg", tag="gmneg")
            nc.vector.tensor_scalar(gmneg[:], gs[:], gs8[:, 3:4], NEG_BIG,
                                    OP.is_lt, OP.mult)
            masked = spool.tile([128, E], F32, name="masked", tag="masked")
            nc.vector.tensor_add(
                masked[:].rearrange("p (g i) -> p g i", g=G),
                sfc[:].rearrange("p (g i) -> p g i", g=G),
                gmneg[:].unsqueeze(2).broadcast_to([128, G, EPG]))
            v8 = spool.tile([128, 8], F32, name="v8", tag="v8")
            nc.vector.max(v8[:], masked[:])
            r = spool.tile([128, E], F32, name="r", tag="r")
            nc.vector.tensor_scalar(r[:], masked[:], v8[:, 7:8], None, OP.is_ge)
            nc.vector.tensor_mul(r[:], r[:], sig[:])
            denom = spool.tile([128, 1], F32, name="denom", tag="denom")
            nc.vector.tensor_reduce(denom[:], r[:], mybir.AxisListType.X, OP.add)
            nc.vector.reciprocal(denom[:], denom[:])
            rn = spool.tile([128, E], F32, name="rn", tag="rn")
            nc.vector.tensor_scalar(rn[:], r[:], denom[:], 2.5, OP.mult, OP.mult)
            nc.sync.dma_start(rn_d[tt * 128:(tt + 1) * 128, :], rn[:])
        pst_ctx.__exit__(None, None, None)

    nc.compile()
    return nc


def _run_device(x, gate, bias, trace=False):
    from concourse.bass_utils import run_bass_kernel_spmd
    if "nc" not in _NC_CACHE:
        _NC_CACHE["nc"] = _build()
    nc = _NC_CACHE["nc"]
    x = np.ascontiguousarray(x, dtype=np.float32)
    gate = np.ascontiguousarray(gate, dtype=np.float32)
    bias = np.ascontiguousarray(bias, dtype=np.float32)
    in_maps = [{"x": np.ascontiguousarray(x[c * T:(c + 1) * T]),
                "gate": gate, "bias": bias} for c in range(N_CORES)]
    res = run_bass_kernel_spmd(nc, in_maps, core_ids=list(range(N_CORES)),
                               trace=trace)
    R = np.concatenate([res.results[c]["rn"] for c in range(N_CORES)], axis=0)
    return R, res


def _assemble(R):
    mask = R > 0.0
    num_tokens_per_expert = mask.sum(axis=0).astype(np.int32)
    maskT = mask.T                      # [E, N] — row-major scan = stable order
    token_idxs = np.nonzero(maskT)[1].astype(np.int32)
    scores = R.T[maskT].astype(np.float32)
    return scores, token_idxs, num_tokens_per_expert


def kernel(x, gate, bias):
    R, _ = _run_device(x, gate, bias, trace=False)
    return _assemble(R)


def kernel_traced(x, gate, bias):
    """kernel() + (R, BassKernelResults) for profiling in test.py."""
    R, res = _run_device(x, gate, bias, trace=True)
    return _assemble(R), R, res


# revision 9
# speedup vs baseline: 1.0232x; 1.0232x over previous
"""DeepSeekV3 token-choice top-k MoE router on 8 Trainium2 NeuronCores.

kernel(x, gate, bias) -> (scores_per_expert [131072] f32,
                          token_idxs_experts_sorted [131072] i32,
                          num_tokens_per_expert [256] i32)

Sharding: x [16384, 7168] split into 8 token shards of 2048; gate/bias
replicated. Each core computes, for its tokens, the dense routing map
rn [2048, 256] f32 (2.5 * normalized unbiased score at the 8 selected
experts, exactly 0 elsewhere). Host gathers shards and assembles the
expert-sorted outputs (stable order == expert-major, token-ascending,
which np.nonzero on the transposed mask yields directly).

Per-core kernel:
  - logitsT[e, t] = gate @ x_shard.T accumulated in PSUM via bf16x3
    (x = xh + xl, gate = gh + gl in bf16; terms xh@gh + xh@gl + xl@gh;
    products are exact in fp32 PSUM, dropped xl@gl ~2^-18 — verified to
    reproduce fp32 top-k selections exactly on this input).
  - x transposed on-chip: 32x32-block-strided DMA (128B runs) + DVE
    32x32 block transpose; bf16 split on GPSIMD (hi) + DVE (lo).
  - Gate is the PE stationary operand: one ldweights per (chunk, half,
    term) reused across 4 moving tiles of 512 tokens. PSUM = 8 banks.
  - sigma = 1/(1 + exp(-l)): ACT Exp (<=2 ulp) + DVE reciprocal.
  - Group-limited top-8 via DVE max8: top-2-per-group sums -> group
    scores; top-4 group threshold -> -1e30 mask; top-8 threshold (8th
    max) -> selection mask; normalize, scale by 2.5.
"""
import numpy as np

N_TOKENS, DIM, NUM_EXPERTS, N_CORES = 16384, 7168, 256, 8
T = N_TOKENS // N_CORES

_NC_CACHE = {}


def _build():
    from contextlib import ExitStack
    import concourse.bacc as bacc
    import concourse.tile as tile
    from concourse import mybir

    F32 = mybir.dt.float32
    BF16 = mybir.dt.bfloat16
    I32 = mybir.dt.int32
    AF = mybir.ActivationFunctionType
    OP = mybir.AluOpType
    NEG_BIG = -1.0e30

    D, E = DIM, NUM_EXPERTS
    KC = D // 128
    NT4 = T // 512
    NT1 = T // 128
    EH = E // 128
    G = 8
    EPG = E // G

    nc = bacc.Bacc("TRN2", target_bir_lowering=False, debug=False,
                   enable_asserts=True)
    x_d = nc.dram_tensor("x", [T, D], F32, kind="ExternalInput")
    g_d = nc.dram_tensor("gate", [E, D], F32, kind="ExternalInput")
    b_d = nc.dram_tensor("bias", [E], F32, kind="ExternalInput")
    rn_d = nc.dram_tensor("rn", [T, E], F32, kind="ExternalOutput")

    with tile.TileContext(nc) as tc, ExitStack() as ctx:
        const = ctx.enter_context(tc.tile_pool(name="const", bufs=1))
        gpool = ctx.enter_context(tc.tile_pool(name="gpool", bufs=1))
        xpool = ctx.enter_context(tc.tile_pool(name="xpool", bufs=4))
        upool = ctx.enter_context(tc.tile_pool(name="upool", bufs=1))
        spool = ctx.enter_context(tc.tile_pool(name="spool", bufs=2))

        bias_bc = const.tile([128, E], F32, name="bias_bc")
        nc.sync.dma_start(bias_bc[0:1, :], b_d[:].unsqueeze(0))
        nc.gpsimd.partition_broadcast(bias_bc[:], bias_bc[0:1, :])

        iot = const.tile([128, 128], I32, name="iot")
        nc.gpsimd.iota(iot[:], pattern=[[-1, 128]], base=0, channel_multiplier=1)
        ident = const.tile([128, 128], F32, name="ident")
        nc.vector.tensor_single_scalar(ident[:], iot[:], 0, OP.is_equal)

        # gate: block-strided load + 32x32 transpose -> gT [dim, e]; bf16 split
        ghs, gls = [], []
        for k in range(KC):
            ga = gpool.tile([128, E], F32, name=f"ga{k}", tag="ga")
            for a in range(4):
                asrc = g_d[:, k * 128 + a * 32:k * 128 + (a + 1) * 32].rearrange(
                    "(b i) j -> i b j", i=32)
                nc.sync.dma_start(
                    ga[a * 32:(a + 1) * 32, :].rearrange("p (b j) -> p b j", j=32),
                    asrc)
            gt = gpool.tile([128, E], F32, name=f"gt{k}", tag="gt")
            nc.vector.transpose(gt[:], ga[:])
            gh = gpool.tile([128, E], BF16, name=f"gh{k}", tag=f"gh{k}")
            nc.vector.tensor_copy(gh[:], gt[:])
            gl = gpool.tile([128, E], BF16, name=f"gl{k}", tag=f"gl{k}")
            nc.vector.tensor_sub(gl[:], gt[:], gh[:])
            ghs.append(gh); gls.append(gl)

        ps_ctx = tc.tile_pool(name="ps", bufs=1, space="PSUM")
        ps = ps_ctx.__enter__()
        psums = [ps.tile([128, 512], F32, name=f"psum{h}_{t}", tag=f"psum{h}_{t}")
                 for h in range(EH) for t in range(NT4)]

        for k in range(KC):
            xa = xpool.tile([128, T], F32, name="xa", tag="xa")
            for a in range(4):
                asrc = x_d[:, k * 128 + a * 32:k * 128 + (a + 1) * 32].rearrange(
                    "(b i) j -> i b j", i=32)
                eng = (nc.sync, nc.sync, nc.scalar, nc.gpsimd)[a]
                eng.dma_start(
                    xa[a * 32:(a + 1) * 32, :].rearrange("p (b j) -> p b j", j=32),
                    asrc)
            xt = xpool.tile([128, T], F32, name="xt", tag="xt")
            nc.vector.transpose(xt[:], xa[:])
            xh = xpool.tile([128, T], BF16, name="xh", tag="xh")
            nc.scalar.copy(xh[:], xt[:])
            xl = xpool.tile([128, T], BF16, name="xl", tag="xl")
            nc.vector.tensor_sub(xl[:], xt[:], xh[:])

            for h in range(EH):
                wh = ghs[k][:, h * 128:(h + 1) * 128]
                nc.tensor.ldweights(wh)
                for xw, first in ((xh, True), (xl, False)):
                    for t in range(NT4):
                        nc.tensor.matmul(
                            psums[h * NT4 + t][:], wh, xw[:, t * 512:(t + 1) * 512],
                            start=(k == 0 and first),
                            stop=False)
                wl = gls[k][:, h * 128:(h + 1) * 128]
                nc.tensor.ldweights(wl)
                for t in range(NT4):
                    nc.tensor.matmul(
                        psums[h * NT4 + t][:], wl, xh[:, t * 512:(t + 1) * 512],
                        start=False,
                        stop=(k == KC - 1))

        uTs = []
        for h in range(EH):
            uT = upool.tile([128, T], F32, name=f"uT{h}", tag=f"uT{h}")
            for t in range(NT4):
                nc.scalar.activation(uT[:, t * 512:(t + 1) * 512],
                                     psums[h * NT4 + t][:], AF.Exp, scale=-1.0)
            uTs.append(uT)
        ps_ctx.__exit__(None, None, None)
        pst_ctx = tc.tile_pool(name="pst", bufs=2, space="PSUM")
        pst = pst_ctx.__enter__()

        for tt in range(NT1):
            u = spool.tile([128, E], F32, name="u", tag="u")
            for h in range(EH):
                ptr = pst.tile([128, 128], F32, name="ptr", tag="ptr")
                nc.tensor.transpose(ptr[:], uTs[h][:, tt * 128:(tt + 1) * 128],
                                    ident[:])
                nc.vector.tensor_copy(u[:, h * 128:(h + 1) * 128], ptr[:])

            sig = spool.tile([128, E], F32, name="sig", tag="sig")
            nc.vector.tensor_scalar_add(sig[:], u[:], 1.0)
            nc.vector.reciprocal(sig[:], sig[:])
            sfc = spool.tile([128, E], F32, name="sfc", tag="sfc")
            nc.vector.tensor_add(sfc[:], sig[:], bias_bc[:])

            m8 = spool.tile([128, G, 8], F32, name="m8", tag="m8")
            for g in range(G):
                nc.vector.max(m8[:, g, :], sfc[:, g * EPG:(g + 1) * EPG])
            gs = spool.tile([128, G], F32, name="gs", tag="gs")
            nc.vector.tensor_add(gs[:], m8[:, :, 0], m8[:, :, 1])
            gs8 = spool.tile([128, 8], F32, name="gs8", tag="gs8")
            nc.vector.max(gs8[:], gs[:])
            gmneg = spool.tile([128, G], F32, name="gmneg", tag="gmneg")
            nc.vector.tensor_scalar(gmneg[:], gs[:], gs8[:, 3:4], NEG_BIG,
                                    OP.is_lt, OP.mult)
            masked = spool.tile([128, E], F32, name="masked", tag="masked")
            nc.vector.tensor_add(
                masked[:].rearrange("p (g i) -> p g i", g=G),
                sfc[:].rearrange("p (g i) -> p g i", g=G),
                gmneg[:].unsqueeze(2).broadcast_to([128, G, EPG]))
            v8 = spool.tile([128, 8], F32, name="v8", tag="v8")
            nc.vector.max(v8[:], masked[:])
            r = spool.tile([128, E], F32, name="r", tag="r")
            nc.vector.tensor_scalar(r[:], masked[:], v8[:, 7:8], None, OP.is_ge)
            nc.vector.tensor_mul(r[:], r[:], sig[:])
            denom = spool.tile([128, 1], F32, name="denom", tag="denom")
            nc.vector.tensor_reduce(denom[:], r[:], mybir.AxisListType.X, OP.add)
            nc.vector.reciprocal(denom[:], denom[:])
            rn = spool.tile([128, E], F32, name="rn", tag="rn")
            nc.vector.tensor_scalar(rn[:], r[:], denom[:], 2.5, OP.mult, OP.mult)
            nc.sync.dma_start(rn_d[tt * 128:(tt + 1) * 128, :], rn[:])
        pst_ctx.__exit__(None, None, None)

    nc.compile()
    return nc


def _run_device(x, gate, bias, trace=False):
    from concourse.bass_utils import run_bass_kernel_spmd
    if "nc" not in _NC_CACHE:
        _NC_CACHE["nc"] = _build()
    nc = _NC_CACHE["nc"]
    x = np.ascontiguousarray(x, dtype=np.float32)
    gate = np.ascontiguousarray(gate, dtype=np.float32)
    bias = np.ascontiguousarray(bias, dtype=np.float32)
    in_maps = [{"x": np.ascontiguousarray(x[c * T:(c + 1) * T]),
                "gate": gate, "bias": bias} for c in range(N_CORES)]
    res = run_bass_kernel_spmd(nc, in_maps, core_ids=list(range(N_CORES)),
                               trace=trace)
    R = np.concatenate([res.results[c]["rn"] for c in range(N_CORES)], axis=0)
    return R, res


def _assemble(R):
    mask = R > 0.0
    num_tokens_per_expert = mask.sum(axis=0).astype(np.int32)
    maskT = mask.T                      # [E, N] — row-major scan = stable order
    token_idxs = np.nonzero(maskT)[1].astype(np.int32)
    scores = R.T[maskT].astype(np.float32)
    return scores, token_idxs, num_tokens_per_expert


def kernel(x, gate, bias):
    R, _ = _run_device(x, gate, bias, trace=False)
    return _assemble(R)


def kernel_traced(x, gate, bias):
    """kernel() + (R, BassKernelResults) for profiling in test.py."""
    R, res = _run_device(x, gate, bias, trace=True)
    return _assemble(R), R, res


# revision 10
# speedup vs baseline: 1.1028x; 1.0778x over previous
"""DeepSeekV3 token-choice top-k MoE router on 8 Trainium2 NeuronCores.

kernel(x, gate, bias) -> (scores_per_expert [131072] f32,
                          token_idxs_experts_sorted [131072] i32,
                          num_tokens_per_expert [256] i32)

Sharding: x [16384, 7168] split into 8 token shards of 2048; gate/bias
replicated. Each core computes, for its tokens, the dense routing map
rn [2048, 256] f32 (2.5 * normalized unbiased score at the 8 selected
experts, exactly 0 elsewhere). Host gathers shards and assembles the
expert-sorted outputs (stable order == expert-major, token-ascending,
which np.nonzero on the transposed mask yields directly).

Per-core kernel:
  - logitsT[e, t] = gate @ x_shard.T accumulated in PSUM via bf16x3
    (x = xh + xl, gate = gh + gl in bf16; terms xh@gh + xh@gl + xl@gh;
    products are exact in fp32 PSUM, dropped xl@gl ~2^-18 — verified to
    reproduce fp32 top-k selections exactly on this input).
  - x transposed on-chip: 32x32-block-strided DMA (128B runs) + DVE
    32x32 block transpose; bf16 split on GPSIMD (hi) + DVE (lo).
  - Gate is the PE stationary operand: one ldweights per (chunk, half,
    term) reused across 4 moving tiles of 512 tokens. PSUM = 8 banks.
  - sigma = 1/(1 + exp(-l)): ACT Exp (<=2 ulp) + DVE reciprocal.
  - Group-limited top-8 via DVE max8: top-2-per-group sums -> group
    scores; top-4 group threshold -> -1e30 mask; top-8 threshold (8th
    max) -> selection mask; normalize, scale by 2.5.
"""
import numpy as np

N_TOKENS, DIM, NUM_EXPERTS, N_CORES = 16384, 7168, 256, 8
T = N_TOKENS // N_CORES

_NC_CACHE = {}


def _build():
    from contextlib import ExitStack
    import concourse.bacc as bacc
    import concourse.tile as tile
    from concourse import mybir

    F32 = mybir.dt.float32
    BF16 = mybir.dt.bfloat16
    I32 = mybir.dt.int32
    AF = mybir.ActivationFunctionType
    OP = mybir.AluOpType
    NEG_BIG = -1.0e30

    D, E = DIM, NUM_EXPERTS
    KC = D // 128
    NT4 = T // 512
    NT1 = T // 128
    EH = E // 128
    G = 8
    EPG = E // G

    nc = bacc.Bacc("TRN2", target_bir_lowering=False, debug=False,
                   enable_asserts=True)
    x_d = nc.dram_tensor("x", [T, D], F32, kind="ExternalInput")
    g_d = nc.dram_tensor("gate", [E, D], F32, kind="ExternalInput")
    b_d = nc.dram_tensor("bias", [E], F32, kind="ExternalInput")
    rn_d = nc.dram_tensor("rn", [T, E], F32, kind="ExternalOutput")

    with tile.TileContext(nc) as tc, ExitStack() as ctx:
        const = ctx.enter_context(tc.tile_pool(name="const", bufs=1))
        gpool = ctx.enter_context(tc.tile_pool(name="gpool", bufs=1))
        xpool = ctx.enter_context(tc.tile_pool(name="xpool", bufs=4))
        upool = ctx.enter_context(tc.tile_pool(name="upool", bufs=1))
        spool = ctx.enter_context(tc.tile_pool(name="spool", bufs=2))

        bias_bc = const.tile([128, E], F32, name="bias_bc")
        nc.sync.dma_start(bias_bc[0:1, :], b_d[:].unsqueeze(0))
        nc.gpsimd.partition_broadcast(bias_bc[:], bias_bc[0:1, :])

        iot = const.tile([128, 128], I32, name="iot")
        nc.gpsimd.iota(iot[:], pattern=[[-1, 128]], base=0, channel_multiplier=1)
        ident = const.tile([128, 128], F32, name="ident")
        nc.vector.tensor_single_scalar(ident[:], iot[:], 0, OP.is_equal)

        # gate: block-strided load + 32x32 transpose -> gT [dim, e]; bf16 split
        ghs, gls = [], []
        for k in range(KC):
            ga = gpool.tile([128, E], F32, name=f"ga{k}", tag="ga")
            for a in range(4):
                asrc = g_d[:, k * 128 + a * 32:k * 128 + (a + 1) * 32].rearrange(
                    "(b i) j -> i b j", i=32)
                nc.sync.dma_start(
                    ga[a * 32:(a + 1) * 32, :].rearrange("p (b j) -> p b j", j=32),
                    asrc)
            gt = gpool.tile([128, E], F32, name=f"gt{k}", tag="gt")
            nc.vector.transpose(gt[:], ga[:])
            gh = gpool.tile([128, E], BF16, name=f"gh{k}", tag=f"gh{k}")
            nc.vector.tensor_copy(gh[:], gt[:])
            gl = gpool.tile([128, E], BF16, name=f"gl{k}", tag=f"gl{k}")
            nc.vector.tensor_sub(gl[:], gt[:], gh[:])
            ghs.append(gh); gls.append(gl)

        ps_ctx = tc.tile_pool(name="ps", bufs=1, space="PSUM")
        ps = ps_ctx.__enter__()
        psums = [ps.tile([128, 512], F32, name=f"psum{h}_{t}", tag=f"psum{h}_{t}")
                 for h in range(EH) for t in range(NT4)]

        for k in range(KC):
            xa = xpool.tile([128, T], F32, name="xa", tag="xa")
            for a in range(4):
                asrc = x_d[:, k * 128 + a * 32:k * 128 + (a + 1) * 32].rearrange(
                    "(b i) j -> i b j", i=32)
                eng = (nc.sync, nc.sync, nc.scalar, nc.gpsimd)[a]
                eng.dma_start(
                    xa[a * 32:(a + 1) * 32, :].rearrange("p (b j) -> p b j", j=32),
                    asrc)
            HT = T // 2
            xhs_half, xls_half = [], []
            for hf in range(2):
                sl = slice(hf * HT, (hf + 1) * HT)
                xt = xpool.tile([128, HT], F32, name=f"xt{hf}", tag=f"xt{hf}")
                nc.vector.transpose(xt[:], xa[:, sl])
                xhh = xpool.tile([128, HT], BF16, name=f"xh{hf}", tag=f"xh{hf}")
                nc.scalar.copy(xhh[:], xt[:])
                xlh = xpool.tile([128, HT], BF16, name=f"xl{hf}", tag=f"xl{hf}")
                nc.vector.tensor_sub(xlh[:], xt[:], xhh[:])
                xhs_half.append(xhh); xls_half.append(xlh)

            NTH = NT4 // 2
            for h in range(EH):
                wh = ghs[k][:, h * 128:(h + 1) * 128]
                nc.tensor.ldweights(wh)
                for xwh, first in ((xhs_half, True), (xls_half, False)):
                    for t in range(NT4):
                        hf, tl = divmod(t, NTH)
                        nc.tensor.matmul(
                            psums[h * NT4 + t][:], wh,
                            xwh[hf][:, tl * 512:(tl + 1) * 512],
                            start=(k == 0 and first),
                            stop=False)
                wl = gls[k][:, h * 128:(h + 1) * 128]
                nc.tensor.ldweights(wl)
                for t in range(NT4):
                    hf, tl = divmod(t, NTH)
                    nc.tensor.matmul(
                        psums[h * NT4 + t][:], wl,
                        xhs_half[hf][:, tl * 512:(tl + 1) * 512],
                        start=False,
                        stop=(k == KC - 1))

        uTs = []
        for h in range(EH):
            uT = upool.tile([128, T], F32, name=f"uT{h}", tag=f"uT{h}")
            for t in range(NT4):
                nc.scalar.activation(uT[:, t * 512:(t + 1) * 512],
                                     psums[h * NT4 + t][:], AF.Exp, scale=-1.0)
            uTs.append(uT)
        ps_ctx.__exit__(None, None, None)
        pst_ctx = tc.tile_pool(name="pst", bufs=2, space="PSUM")
        pst = pst_ctx.__enter__()

        for tt in range(NT1):
            u = spool.tile([128, E], F32, name="u", tag="u")
            for h in range(EH):
                ptr = pst.tile([128, 128], F32, name="ptr", tag="ptr")
                nc.tensor.transpose(ptr[:], uTs[h][:, tt * 128:(tt + 1) * 128],
                                    ident[:])
                nc.vector.tensor_copy(u[:, h * 128:(h + 1) * 128], ptr[:])

            sig = spool.tile([128, E], F32, name="sig", tag="sig")
            nc.vector.tensor_scalar_add(sig[:], u[:], 1.0)
            nc.vector.reciprocal(sig[:], sig[:])
            sfc = spool.tile([128, E], F32, name="sfc", tag="sfc")
            nc.vector.tensor_add(sfc[:], sig[:], bias_bc[:])

            m8 = spool.tile([128, G, 8], F32, name="m8", tag="m8")
            for g in range(G):
                nc.vector.max(m8[:, g, :], sfc[:, g * EPG:(g + 1) * EPG])
            gs = spool.tile([128, G], F32, name="gs", tag="gs")
            nc.vector.tensor_add(gs[:], m8[:, :, 0], m8[:, :, 1])
            gs8 = spool.tile([128, 8], F32, name="gs8", tag="gs8")
            nc.vector.max(gs8[:], gs[:])
            gmneg = spool.tile([128, G], F32, name="gmneg", tag="gmneg")
            nc.vector.tensor_scalar(gmneg[:], gs[:], gs8[:, 3:4], NEG_BIG,
                                    OP.is_lt, OP.mult)
            masked = spool.tile([128, E], F32, name="masked", tag="masked")
            nc.vector.tensor_add(
                masked[:].rearrange("p (g i) -> p g i", g=G),
                sfc[:].rearrange("p (g i) -> p g i", g=G),
                gmneg[:].unsqueeze(2).broadcast_to([128, G, EPG]))
            v8 = spool.tile([128, 8], F32, name="v8", tag="v8")
            nc.vector.max(v8[:], masked[:])
            r = spool.tile([128, E], F32, name="r", tag="r")
            nc.vector.tensor_scalar(r[:], masked[:], v8[:, 7:8], None, OP.is_ge)
            nc.vector.tensor_mul(r[:], r[:], sig[:])
            denom = spool.tile([128, 1], F32, name="denom", tag="denom")
            nc.vector.tensor_reduce(denom[:], r[:], mybir.AxisListType.X, OP.add)
            nc.vector.reciprocal(denom[:], denom[:])
            rn = spool.tile([128, E], F32, name="rn", tag="rn")
            nc.vector.tensor_scalar(rn[:], r[:], denom[:], 2.5, OP.mult, OP.mult)
            nc.sync.dma_start(rn_d[tt * 128:(tt + 1) * 128, :], rn[:])
        pst_ctx.__exit__(None, None, None)

    nc.compile()
    return nc


def _run_device(x, gate, bias, trace=False):
    from concourse.bass_utils import run_bass_kernel_spmd
    if "nc" not in _NC_CACHE:
        _NC_CACHE["nc"] = _build()
    nc = _NC_CACHE["nc"]
    x = np.ascontiguousarray(x, dtype=np.float32)
    gate = np.ascontiguousarray(gate, dtype=np.float32)
    bias = np.ascontiguousarray(bias, dtype=np.float32)
    in_maps = [{"x": np.ascontiguousarray(x[c * T:(c + 1) * T]),
                "gate": gate, "bias": bias} for c in range(N_CORES)]
    res = run_bass_kernel_spmd(nc, in_maps, core_ids=list(range(N_CORES)),
                               trace=trace)
    R = np.concatenate([res.results[c]["rn"] for c in range(N_CORES)], axis=0)
    return R, res


def _assemble(R):
    mask = R > 0.0
    num_tokens_per_expert = mask.sum(axis=0).astype(np.int32)
    maskT = mask.T                      # [E, N] — row-major scan = stable order
    token_idxs = np.nonzero(maskT)[1].astype(np.int32)
    scores = R.T[maskT].astype(np.float32)
    return scores, token_idxs, num_tokens_per_expert


def kernel(x, gate, bias):
    R, _ = _run_device(x, gate, bias, trace=False)
    return _assemble(R)


def kernel_traced(x, gate, bias):
    """kernel() + (R, BassKernelResults) for profiling in test.py."""
    R, res = _run_device(x, gate, bias, trace=True)
    return _assemble(R), R, res
